# revision 1
# baseline (speedup 1.0000x reference)
"""Multi-head attention (16 heads, D=1024, B=2, S=2048) on 8 Trainium2 cores.

Sharding: batch (2) x head-groups (4 heads each) = 8 cores, no collectives.
Each core computes, for its batch b and head group g:
  - Q/K/V projections restricted to the group's 256 output dims
  - per-head attention with masked softmax (mask + 1/sqrt(32) scale folded
    into a single exp activation; no max-subtraction needed since scores are
    small and bounded)
  - partial output = concat(head outs) @ Wo[rows of group g]
Host sums the 4 per-group partials for each batch.

Device layout trick: the host passes X^T (feature-major) so every matmul
chains naturally with zero on-device transposes:
  X^T --(Wq/Wk stationary)--> Q^T,K^T [j, s]
  K^T.T @ Q^T = scores^T [k, q]  --exp-->  E^T
  V is produced in natural [s, j] layout with an interleaved ones column,
  so V'.T @ E^T accumulates attn-weighted V AND the softmax denominator
  (row 64) in one PSUM accumulation group.

All matmuls run in float32r (the PE's 1-cycle/row fp32 mode; plain fp32 is
4 cycles/row). Projections are emitted in 4 interleaved K/Q/V column-rounds
and K^T/Q^T/Oc^T are split into per-chunk tiles so attention/output phases
start as soon as their actual inputs exist.
"""
import ml_dtypes
import numpy as np

import concourse.bacc as bacc
import concourse.mybir as mybir
import concourse.tile as tile
from concourse.bass_utils import run_bass_kernel_spmd

F32 = mybir.dt.float32
F32R = mybir.dt.float32r
BF16 = mybir.dt.bfloat16
AF = mybir.ActivationFunctionType

S = 2048          # sequence length
D = 1024          # model dim
HLOC = 4          # heads per core
HD = 64           # head dim
JG = HLOC * 65    # V-natural tile width (64 data cols + 1 ones col per head)
SCALE = 1.0 / np.sqrt(32.0)   # reference bug: d_k = B*H = 32
MASK_VALUE = -1.0e6

ND = 8            # d chunks of 128 (contraction for projections)
NSC = 4           # s chunks of 512 (projection rounds)
NST = 16          # s tiles of 128
NKC = 16          # k chunks of 128
NQC = 2           # q chunks of 1024
QW = 1024         # q chunk width

_cached_nc = None
LAST_RESULTS = None


def _build():
    nc = bacc.Bacc("TRN2", target_bir_lowering=False, debug=False,
                   num_swdge_queues=4)

    xqT = nc.dram_tensor("xqT", [D, S], BF16, kind="ExternalInput")
    xkT = nc.dram_tensor("xkT", [D, S], BF16, kind="ExternalInput")
    xvT = nc.dram_tensor("xvT", [D, S], BF16, kind="ExternalInput")
    wq = nc.dram_tensor("wq", [D, 256], BF16, kind="ExternalInput")
    wk = nc.dram_tensor("wk", [D, 256], BF16, kind="ExternalInput")
    wv = nc.dram_tensor("wv", [D, 256], BF16, kind="ExternalInput")
    wo = nc.dram_tensor("wo", [256, D], F32R, kind="ExternalInput")
    maskb = nc.dram_tensor("maskb", [128, NKC], F32, kind="ExternalInput")
    out = nc.dram_tensor("out", [S, D], F32, kind="ExternalOutput")

    with tile.TileContext(nc) as tc:
        with tc.tile_pool(name="wp", bufs=1) as wp, \
             tc.tile_pool(name="per", bufs=1) as per, \
             tc.tile_pool(name="xp", bufs=16) as xp, \
             tc.tile_pool(name="ep", bufs=4) as ep, \
             tc.tile_pool(name="unp", bufs=8) as unp, \
             tc.tile_pool(name="rbp", bufs=8) as rbp, \
             tc.tile_pool(name="smol", bufs=1) as smol, \
             tc.tile_pool(name="outp", bufs=6) as outp, \
             tc.tile_pool(name="pj", bufs=2, space="PSUM") as pj, \
             tc.tile_pool(name="psc", bufs=2, space="PSUM") as psc, \
             tc.tile_pool(name="po", bufs=2, space="PSUM") as po:

            # ---- mask + packed projection weights (one 1MB DMA per W) ----
            mt = wp.tile([128, NKC], F32, name="mt", tag="mt")
            nc.sync.dma_start(out=mt, in_=maskb[:, :])
            wk_p = wp.tile([128, ND * 256], BF16, name="wk_p", tag="wk_p")
            wq_p = wp.tile([128, ND * 256], BF16, name="wq_p", tag="wq_p")
            wv_p = wp.tile([128, ND * 256], BF16, name="wv_p", tag="wv_p")
            nc.sync.dma_start(out=wk_p.rearrange("p (n j) -> p n j", j=256),
                              in_=wk.rearrange("(n p) j -> p n j", p=128))
            nc.gpsimd.dma_start(out=wv_p.rearrange("p (n j) -> p n j", j=256),
                                in_=wv.rearrange("(n p) j -> p n j", p=128))
            wk_t = [wk_p[:, d * 256:(d + 1) * 256] for d in range(ND)]
            wq_t = [wq_p[:, d * 256:(d + 1) * 256] for d in range(ND)]
            wv_t = [wv_p[:, d * 256:(d + 1) * 256] for d in range(ND)]
            # exp table preload: a 1-element exp so the ~2.7us ACT table
            # load happens during the projection lead-in, not mid-pipeline
            scr1 = wp.tile([1, 1], F32, name="scr1", tag="scr1")
            nc.scalar.activation(scr1, mt[0:1, 0:1], AF.Exp)

            # ---- persistent activations (chunked for dep granularity) ----
            KTt = [[per.tile([128, 512], F32R, name=f"KT{j}_{s_}",
                             tag=f"KT{j}_{s_}") for s_ in range(NSC)]
                   for j in range(2)]
            QTt = [[per.tile([128, 512], F32R, name=f"QT{j}_{s_}",
                             tag=f"QT{j}_{s_}") for s_ in range(NSC)]
                   for j in range(2)]
            Vn = [per.tile([128, JG], F32R, name=f"Vn{i}", tag=f"Vn{i}")
                  for i in range(NST)]
            OcT = [[per.tile([128, 512], F32R, name=f"OcT{j}_{q}",
                             tag=f"OcT{j}_{q}") for q in range(2 * NQC)]
                   for j in range(2)]

            def k_or_q_round(nm, xdram, wt, OUT, r):
                c0 = r * 512
                xt = [xp.tile([128, 512], BF16, name=f"x{nm}{r}_{d}",
                              tag="xin") for d in range(ND)]
                for d in range(ND):
                    nc.sync.dma_start(
                        out=xt[d],
                        in_=xdram[d * 128:(d + 1) * 128, c0:c0 + 512])
                for jt in range(2):
                    pt = pj.tile([128, 512], F32, name=f"p{nm}{r}_{jt}",
                                 tag="pj")
                    for d in range(ND):
                        nc.tensor.matmul(
                            pt, wt[d][:, jt * 128:(jt + 1) * 128],
                            xt[d], start=(d == 0), stop=(d == ND - 1))
                    nc.vector.tensor_copy(OUT[jt][r], pt)

            def v_round(r):
                c0 = r * 512
                xvt = [xp.tile([128, 512], BF16, name=f"xv{r}_{d}", tag="xin")
                       for d in range(ND)]
                for d in range(ND):
                    nc.gpsimd.dma_start(
                        out=xvt[d],
                        in_=xvT[d * 128:(d + 1) * 128, c0:c0 + 512])
                for stl in range(4):
                    st = r * 4 + stl
                    pt = pj.tile([128, 256], F32, name=f"pv{st}", tag="pj")
                    for d in range(ND):
                        nc.tensor.matmul(
                            pt, xvt[d][:, stl * 128:(stl + 1) * 128], wv_t[d],
                            start=(d == 0), stop=(d == ND - 1))
                    vt = Vn[st]
                    vspl = vt.rearrange("p (h x) -> p h x", x=65)
                    nc.vector.memset(vspl[:, :, 64:65].bitcast(F32), 1.0)
                    nc.vector.tensor_copy(
                        vspl[:, :, 0:64],
                        pt.rearrange("p (h j) -> p h j", j=64))

            def attention_head(qc, h, seg_hook=None, tail_head=False):
                jt, hr = divmod(h, 2)
                hoff = hr * 64
                pots = [po.tile([65, 512], F32, name=f"pot{qc}_{h}_{qh}",
                                tag="pot") for qh in range(2)]
                for kc in range(NKC):
                    if seg_hook is not None and kc % 4 == 0:
                        seg_hook(kc)
                    ks, ko = divmod(kc, 4)
                    pst = psc.tile([128, QW], F32,
                                   name=f"pst{qc}_{h}_{kc}", tag="pst")
                    for qh in range(2):
                        nc.tensor.matmul(
                            pst[:, qh * 512:(qh + 1) * 512],
                            KTt[jt][ks][hoff:hoff + 64,
                                        ko * 128:(ko + 1) * 128],
                            QTt[jt][2 * qc + qh][hoff:hoff + 64, :],
                            start=True, stop=True)
                    et = ep.tile([128, QW], F32R,
                                 name=f"et{qc}_{h}_{kc}", tag="et")
                    nc.scalar.activation(et, pst, AF.Exp,
                                         bias=mt[:, kc:kc + 1],
                                         scale=float(SCALE))
                    for qh in range(2):
                        nc.tensor.matmul(
                            pots[qh],
                            Vn[kc][:, h * 65:h * 65 + 65],
                            et[:, qh * 512:(qh + 1) * 512],
                            start=(kc == 0), stop=(kc == NKC - 1))
                # drain + normalize each q-half independently: the first
                # half's chain (and its PSUM bank) overlaps the second
                # half's tail, and the output projection unblocks per half
                for qh in range(2):
                    un = unp.tile([65, 512], F32, name=f"un{qc}_{h}_{qh}",
                                  tag="un")
                    dtmp = rbp.tile([1, 512], F32, name=f"dt{qc}_{h}_{qh}",
                                    tag="tmp1")
                    if tail_head:
                        # very last head: ACT is idle, so drain on ACT while
                        # DVE stages the denom row straight from PSUM --
                        # shortens the serial chain before the final wo tiles
                        nc.scalar.copy(un, pots[qh][:, :])
                        nc.vector.tensor_copy(dtmp, pots[qh][64:65, :])
                    else:
                        nc.vector.tensor_copy(un, pots[qh][:, :])
                        # reciprocal_approx_* reads garbage at a nonzero
                        # partition offset: stage the denom row at part. 0
                        nc.vector.tensor_copy(dtmp, un[64:65, :])
                    rrow = rbp.tile([1, 512], F32, name=f"rr{qc}_{h}_{qh}",
                                    tag="tmp1")
                    rsc1 = rbp.tile([1, 512], F32, name=f"rs{qc}_{h}_{qh}",
                                    tag="tmp1")
                    nc.vector.reciprocal_approx_accurate(rrow, dtmp, rsc1)
                    rb = rbp.tile([64, 512], F32, name=f"rb{qc}_{h}_{qh}",
                                  tag="rb")
                    nc.gpsimd.partition_broadcast(rb, rrow[0:1, :])
                    nc.vector.tensor_mul(
                        OcT[jt][2 * qc + qh][hoff:hoff + 64, :],
                        un[0:64, :], rb)

            def wo_phase(sts, tail):
                for i, st in enumerate(sts):
                    sq, so = divmod(st, 4)
                    for ec in range(2):
                        pool = psc if (tail and (i + ec) % 2 == 0) else pj
                        ptag = "pst" if pool is psc else "pj"
                        pt = pool.tile([128, 512], F32, name=f"pw{st}_{ec}",
                                       tag=ptag)
                        for jc in range(2):
                            nc.tensor.matmul(
                                pt, OcT[jc][sq][:, so * 128:(so + 1) * 128],
                                wo_t[jc][:, ec * 512:(ec + 1) * 512],
                                start=(jc == 0), stop=(jc == 1))
                        ot = outp.tile([128, 512], F32, name=f"ot{st}_{ec}",
                                       tag="ot")
                        if tail and ec == 0:
                            nc.scalar.copy(ot, pt)
                        else:
                            nc.vector.tensor_copy(ot, pt)
                        nc.sync.dma_start(
                            out=out[st * 128:(st + 1) * 128,
                                    ec * 512:(ec + 1) * 512],
                            in_=ot)

            # ---- emission schedule ----
            # lead-in: exactly what attention(qc0, h0, kc0..3) needs, first
            k_or_q_round("k", xkT, wk_t, KTt, 0)
            nc.sync.dma_start(out=wq_p.rearrange("p (n j) -> p n j", j=256),
                              in_=wq.rearrange("(n p) j -> p n j", p=128))
            k_or_q_round("q", xqT, wq_t, QTt, 0)
            k_or_q_round("q", xqT, wq_t, QTt, 1)
            v_round(0)

            def h0_hook(kc):
                # stream the remaining K/V rounds in just ahead of the
                # segments of head 0 that consume them
                if kc == 4:
                    k_or_q_round("k", xkT, wk_t, KTt, 1)
                    v_round(1)
                elif kc == 8:
                    k_or_q_round("k", xkT, wk_t, KTt, 2)
                    v_round(2)
                elif kc == 12:
                    k_or_q_round("k", xkT, wk_t, KTt, 3)
                    v_round(3)

            attention_head(0, 0, seg_hook=h0_hook)
            for h in range(1, HLOC):
                attention_head(0, h)

            wo_p = wp.tile([128, 2 * D], F32R, name="wo_p", tag="wo_p")
            nc.sync.dma_start(out=wo_p.rearrange("p (n j) -> p n j", j=D),
                              in_=wo.rearrange("(n p) j -> p n j", p=128))
            wo_t = [wo_p[:, j * D:(j + 1) * D] for j in range(2)]

            k_or_q_round("q", xqT, wq_t, QTt, 2)
            k_or_q_round("q", xqT, wq_t, QTt, 3)
            for h in range(HLOC):
                attention_head(1, h, tail_head=(h == HLOC - 1))
            wo_phase(range(0, 8), False)   # qc0: runs under attention(qc1)
            wo_phase(range(8, 16), True)   # qc1: tail, ACT idle, more psum
    nc.compile()
    return nc


def _get_nc():
    global _cached_nc
    if _cached_nc is None:
        _cached_nc = _build()
    return _cached_nc


def kernel(queries, keys, values, valid_lens, Wq, Wk, Wv, Wo, **kwargs):
    queries = np.asarray(queries, dtype=np.float32)
    keys = np.asarray(keys, dtype=np.float32)
    values = np.asarray(values, dtype=np.float32)
    Wq = np.asarray(Wq, dtype=np.float32)
    Wk = np.asarray(Wk, dtype=np.float32)
    Wv = np.asarray(Wv, dtype=np.float32)
    Wo = np.asarray(Wo, dtype=np.float32)
    vls = np.asarray(valid_lens).astype(np.int64)
    B = queries.shape[0]
    assert B == 2 and queries.shape[1:] == (S, D), \
        f"kernel compiled for (2, {S}, {D}), got {queries.shape}"

    nc = _get_nc()

    in_maps = []
    for b in range(B):
        vl = int(vls[b])
        qb = queries[b]
        if vl <= 0:
            # reference: fully-masked row -> softmax of constant -> uniform.
            # Zero queries give zero scores -> uniform attention, and an
            # all-zero mask keeps every position in the denominator.
            qb = np.zeros_like(qb)
            mk = np.zeros(S, np.float32)
        else:
            mk = np.where(np.arange(S) < vl, 0.0, MASK_VALUE).astype(np.float32)
        mkt = np.ascontiguousarray(mk.reshape(NKC, 128).T)  # [128, NKC]
        bf16 = ml_dtypes.bfloat16
        xq = np.ascontiguousarray(qb.T).astype(bf16)
        xk = np.ascontiguousarray(keys[b].T).astype(bf16)
        xv = np.ascontiguousarray(values[b].T).astype(bf16)
        for g in range(4):
            in_maps.append({
                "xqT": xq, "xkT": xk, "xvT": xv,
                "wq": np.ascontiguousarray(Wq[:, g * 256:(g + 1) * 256]).astype(bf16),
                "wk": np.ascontiguousarray(Wk[:, g * 256:(g + 1) * 256]).astype(bf16),
                "wv": np.ascontiguousarray(Wv[:, g * 256:(g + 1) * 256]).astype(bf16),
                "wo": np.ascontiguousarray(Wo[g * 256:(g + 1) * 256, :]),
                "maskb": mkt,
            })

    res = run_bass_kernel_spmd(nc, in_maps, core_ids=list(range(8)), **kwargs)
    global LAST_RESULTS
    LAST_RESULTS = res

    outp = np.zeros((B, S, D), np.float32)
    for b in range(B):
        acc = res.results[b * 4 + 0]["out"].astype(np.float32)
        for g in range(1, 4):
            acc = acc + res.results[b * 4 + g]["out"]
        outp[b] = acc
    return outp



# revision 11
# speedup vs baseline: 2.1511x; 2.1511x over previous
"""Multi-head attention (16 heads, D=1024, B=2, S=2048) on 8 Trainium2 cores.

Sharding: batch (2) x head-groups (4 heads each) = 8 cores, no collectives.
Each core computes, for its batch b and head group g:
  - Q projection for all 2048 positions, K/V projections only for the
    valid-key extent (valid_lens specializes the compiled program: fully
    masked key chunks are never computed - exp() would zero them anyway)
  - per-head attention with masked softmax over the valid chunks only
  - partial output = concat(head outs) @ Wo[rows of group g]
Host sums the 4 per-group partials for each batch.

The program is compiled for NKC = ceil(max(valid_lens)/128) key chunks and
cached per NKC; per-batch masks remain runtime data so one program serves
both batches (a batch with smaller vl just sees more masked columns).

Attention data layout:
  X^T (feature-major, host-transposed) --(Wq/Wk stationary)--> Q^T,K^T [j,s]
  K^T chunk (stationary) x Q^T (moving)  -> scores^T [k,q] in PSUM
  exp(scale*scores + mask) -> E [k,q] bf16
  AV is "flipped" for full PE utilization (128-deep contraction over k,
  128 output partitions over q): stationary E [k,128q] x moving V [k,64j]
  accumulated over k chunks; softmax denominators accumulate in a separate
  PSUM bank via 1-column matmuls against a ones vector.
  Normalize with per-partition reciprocal * scalar-mul, PE-transpose the
  [q,j] tile to [j,q], then Oc^T (stationary) x Wo rows (moving) -> out.

All matmul moving operands are bf16 (1 cycle/row on the PE at any width).

Because engines execute their instruction streams in order, V/Q projection
rounds and the previous q-chunk's normalize/transpose/Wo work are emitted
as filler tasks BETWEEN attention iterations: they fill the PE bubbles the
scores -> exp(ACT) -> AV round trip would otherwise leave.
"""
import ml_dtypes
import numpy as np

import concourse.bacc as bacc
import concourse.mybir as mybir
import concourse.tile as tile
from concourse.bass_utils import run_bass_kernel_spmd

F32 = mybir.dt.float32
BF16 = mybir.dt.bfloat16
AF = mybir.ActivationFunctionType

S = 2048          # sequence length
D = 1024          # model dim
HLOC = 4          # heads per core
HD = 64           # head dim
JW = HLOC * HD    # 256 output dims per core
SCALE = 1.0 / np.sqrt(32.0)   # reference bug: d_k = B*H = 32
MASK_VALUE = -1.0e6

ND = 8            # d chunks of 128 (contraction for projections)
NQR = 4           # q projection rounds of 512
QW = 512          # q chunk width (scores moving width)
NQC = S // QW     # 4 q chunks of 512
NQI = QW // 128   # 4 q subchunks of 128 per q chunk

_cached_nc = {}
LAST_RESULTS = None


def _build(nkc):
    kw = nkc * 128   # padded valid-key extent
    kr = []          # k-projection rounds of up to 512 columns
    c = 0
    while c < kw:
        w = min(512, kw - c)
        kr.append((c, w))
        c += w

    nc = bacc.Bacc("TRN2", target_bir_lowering=False, debug=False,
                   num_swdge_queues=4)

    xqT = nc.dram_tensor("xqT", [D, S], BF16, kind="ExternalInput")
    xkT = nc.dram_tensor("xkT", [D, kw], BF16, kind="ExternalInput")
    xvT = nc.dram_tensor("xvT", [D, kw], BF16, kind="ExternalInput")
    wq = nc.dram_tensor("wq", [D, JW], BF16, kind="ExternalInput")
    wk = nc.dram_tensor("wk", [D, JW], BF16, kind="ExternalInput")
    wv = nc.dram_tensor("wv", [D, JW], BF16, kind="ExternalInput")
    wo = nc.dram_tensor("wo", [JW, D], BF16, kind="ExternalInput")
    maskb = nc.dram_tensor("maskb", [128, nkc], F32, kind="ExternalInput")
    ident = nc.dram_tensor("ident", [128, 128], BF16, kind="ExternalInput")
    out = nc.dram_tensor("out", [S, D], BF16, kind="ExternalOutput")

    with tile.TileContext(nc) as tc:
        with tc.tile_pool(name="wp", bufs=1) as wp, \
             tc.tile_pool(name="per", bufs=1) as per, \
             tc.tile_pool(name="xp", bufs=2) as xp, \
             tc.tile_pool(name="ep", bufs=6) as ep, \
             tc.tile_pool(name="onp", bufs=4) as onp, \
             tc.tile_pool(name="ocp", bufs=8) as ocp, \
             tc.tile_pool(name="rbp", bufs=4) as rbp, \
             tc.tile_pool(name="outp", bufs=6) as outp, \
             tc.tile_pool(name="psc", bufs=4, space="PSUM") as psc, \
             tc.tile_pool(name="pa", bufs=4, space="PSUM") as pa:

            # ---- mask, identity, packed projection weights ----
            mt = wp.tile([128, nkc], F32, name="mt", tag="mt")
            nc.gpsimd.dma_start(out=mt, in_=maskb[:, :])
            idt = wp.tile([128, 128], BF16, name="idt", tag="idt")
            nc.gpsimd.dma_start(out=idt, in_=ident[:, :])
            onest = wp.tile([128, 1], BF16, name="onest", tag="onest")
            nc.vector.memset(onest, 1.0)

            wk_p = wp.tile([128, ND * JW], BF16, name="wk_p", tag="wk_p")
            wq_p = wp.tile([128, ND * JW], BF16, name="wq_p", tag="wq_p")
            wv_p = wp.tile([128, ND * JW], BF16, name="wv_p", tag="wv_p")
            nc.sync.dma_start(out=wk_p.rearrange("p (n j) -> p n j", j=JW),
                              in_=wk.rearrange("(n p) j -> p n j", p=128))
            nc.gpsimd.dma_start(out=wv_p.rearrange("p (n j) -> p n j", j=JW),
                                in_=wv.rearrange("(n p) j -> p n j", p=128))
            wk_t = [wk_p[:, d * JW:(d + 1) * JW] for d in range(ND)]
            wq_t = [wq_p[:, d * JW:(d + 1) * JW] for d in range(ND)]
            wv_t = [wv_p[:, d * JW:(d + 1) * JW] for d in range(ND)]
            wo_p = wp.tile([128, 2 * D], BF16, name="wo_p", tag="wo_p")
            wo_t = [wo_p[:, j * D:(j + 1) * D] for j in range(2)]
            # exp table preload: a 1-element exp so the ACT table load
            # happens during the projection lead-in, not mid-pipeline
            scr1 = wp.tile([1, 1], F32, name="scr1", tag="scr1")
            nc.scalar.activation(scr1, mt[0:1, 0:1], AF.Exp)

            # ---- persistent activations ----
            KTt = [per.tile([128, kw], BF16, name=f"KT{j}", tag=f"KT{j}")
                   for j in range(2)]
            QTt = [per.tile([128, S], BF16, name=f"QT{j}", tag=f"QT{j}")
                   for j in range(2)]
            Vn = [per.tile([128, HLOC * 65], BF16, name=f"Vn{i}",
                           tag=f"Vn{i}") for i in range(nkc)]

            def xq_round_dma(r):
                xt = xp.tile([128, ND * 512], BF16, name=f"xq{r}",
                             tag="xin", bufs=NQR)
                nc.gpsimd.dma_start(
                    out=xt.rearrange("p (n s) -> p n s", s=512),
                    in_=xqT.rearrange("(n p) s -> p n s", p=128)[
                        :, :, r * 512:(r + 1) * 512])
                return xt

            def proj_half(nm, xt, w, wt, OUT, c0, jt, dlo, dhi, pt_box):
                # half of one projection round: d chunks [dlo,dhi) into the
                # jt-th 128-column tile; copies the PSUM out on the last.
                if dlo == 0:
                    pt_box[jt] = psc.tile([128, w], F32,
                                          name=f"p{nm}{c0}_{jt}", tag="ps",
                                          padded_shape=[128, 512])
                pt = pt_box[jt]
                for d in range(dlo, dhi):
                    nc.tensor.matmul(
                        pt, wt[d][:, jt * 128:(jt + 1) * 128],
                        xt[:, d * w:(d + 1) * w],
                        start=(d == 0), stop=(d == ND - 1))
                if dhi == ND:
                    nc.vector.tensor_copy(OUT[jt][:, c0:c0 + w], pt)

            def k_round(c0, w, split=False):
                halves = [(0, w)]
                if split and w > 256:
                    halves = [(0, w // 2), (w // 2, w - w // 2)]
                xts = []
                for (h0, hw) in halves:
                    xt = xp.tile([128, ND * 512], BF16,
                                 name=f"xk{c0}_{h0}", tag="xkin", bufs=2,
                                 padded_shape=[128, ND * 512])
                    nc.sync.dma_start(
                        out=xt[:, 0:ND * hw].rearrange("p (n s) -> p n s",
                                                       s=hw),
                        in_=xkT.rearrange("(n p) s -> p n s", p=128)[
                            :, :, c0 + h0:c0 + h0 + hw])
                    xts.append((h0, hw, xt))
                for (h0, hw, xt) in xts:
                    box = [None, None]
                    for jt in range(2):
                        proj_half("k", xt, hw, wk_t, KTt, c0 + h0, jt,
                                  0, ND, box)

            def v_dma(sc):
                xvt = xp.tile([128, ND * 128], BF16, name=f"xv{sc}",
                              tag="xvin", bufs=nkc)
                nc.gpsimd.dma_start(
                    out=xvt.rearrange("p (n s) -> p n s", s=128),
                    in_=xvT.rearrange("(n p) s -> p n s", p=128)[
                        :, :, sc * 128:(sc + 1) * 128])
                return xvt

            def v_compute(sc, xvt):
                pv = psc.tile([128, JW], F32, name=f"pv{sc}", tag="ps",
                              padded_shape=[128, 512])
                for d in range(ND):
                    nc.tensor.matmul(
                        pv, xvt[:, d * 128:(d + 1) * 128], wv_t[d],
                        start=(d == 0), stop=(d == ND - 1))
                vspl = Vn[sc].rearrange("p (h x) -> p h x", x=65)
                nc.vector.memset(vspl[:, :, 64:65], 1.0)
                nc.vector.tensor_copy(
                    vspl[:, :, 0:64],
                    pv.rearrange("p (h j) -> p h j", j=64))

            def norm_head(qc, h, Ah, on_box):
                # DVE-only: per-partition reciprocal of the denominator
                # column, scale the head's 64 columns to bf16
                for qi in range(NQI):
                    qcg = qc * NQI + qi
                    rt = rbp.tile([128, 1], F32, name=f"rt{qcg}_{h}",
                                  tag="rt")
                    nc.vector.reciprocal(rt, Ah[qi][:, 64:65])
                    nc.vector.tensor_scalar_mul(
                        on_box[qi][:, h * 64:(h + 1) * 64],
                        Ah[qi][:, 0:64], rt)

            def trans_qi(qc, qi, on_box, oc_box):
                # on_box[qi] written per head during attention(qc)
                qcg = qc * NQI + qi
                oc = []
                for jt in range(2):
                    pt = psc.tile([128, 128], BF16, name=f"ptt{qcg}_{jt}",
                                  tag="ps")
                    nc.tensor.transpose(
                        pt, on_box[qi][:, jt * 128:(jt + 1) * 128], idt)
                    ot = ocp.tile([128, 128], BF16, name=f"oc{qcg}_{jt}",
                                  tag="oc")
                    nc.vector.tensor_copy(ot, pt)
                    oc.append(ot)
                oc_box[qi] = oc

            def wo_qi(qc, qi, dh, oc_box):
                qcg = qc * NQI + qi
                pw = psc.tile([128, 512], F32, name=f"pw{qcg}_{dh}",
                              tag="ps")
                for jt in range(2):
                    nc.tensor.matmul(
                        pw, oc_box[qi][jt],
                        wo_t[jt][:, dh * 512:(dh + 1) * 512],
                        start=(jt == 0), stop=(jt == 1))
                ob = outp.tile([128, 512], BF16, name=f"ob{qcg}_{dh}",
                               tag="ob")
                if qcg >= (NQC - 1) * NQI or (qcg + dh) % 4 == 0:
                    nc.scalar.copy(ob, pw)
                else:
                    nc.vector.tensor_copy(ob, pw)
                nc.sync.dma_start(
                    out=out[qcg * 128:(qcg + 1) * 128,
                            dh * 512:(dh + 1) * 512],
                    in_=ob)

            def emit_av(qc, st_box, h, kc, et):
                st = (kc == 0)
                sp = (kc == nkc - 1)
                if st:
                    st_box[h] = [
                        pa.tile([128, 65], F32, name=f"A{qc}_{h}_{qi}",
                                tag="pa") for qi in range(NQI)]
                Ah = st_box[h]
                for qi in range(NQI):
                    nc.tensor.matmul(
                        Ah[qi], et[:, qi * 128:(qi + 1) * 128],
                        Vn[kc][:, h * 65:(h + 1) * 65],
                        start=st, stop=sp)
                if sp:
                    norm_head(qc, h, Ah, st_box["on"])

            def attention(qc, st_box, fillers):
                # iteration order (h, kc); AV(i) is emitted two iterations
                # late so filler PE work and scores(i+1..2) hide exp(i)'s
                # ACT round trip. Fillers are (pe_ns, fn): each iteration
                # pops tasks until the PE has ~the exp() duration queued.
                fillers = list(fillers)
                debt = 0.0
                pending = []
                for h in range(HLOC):
                    jt, hr = divmod(h, 2)
                    hb = hr * 64
                    for kc in range(nkc):
                        pst = psc.tile([128, QW], F32,
                                       name=f"pst{qc}_{h}_{kc}", tag="ps")
                        nc.tensor.matmul(
                            pst,
                            KTt[jt][hb:hb + 64, kc * 128:(kc + 1) * 128],
                            QTt[jt][hb:hb + 64, qc * QW:(qc + 1) * QW],
                            start=True, stop=True)
                        et = ep.tile([128, QW], BF16,
                                     name=f"et{qc}_{h}_{kc}", tag="et")
                        nc.scalar.activation(et, pst, AF.Exp,
                                             bias=mt[:, kc:kc + 1],
                                             scale=float(SCALE))
                        if len(pending) >= 3:
                            emit_av(qc, st_box, *pending.pop(0))
                        debt += 280.0   # exp pacing minus scores+AV work
                        while debt > 0 and fillers:
                            pe_ns, fn = fillers.pop(0)
                            fn()
                            debt -= pe_ns
                        pending.append((h, kc, et))
                for p in pending:
                    emit_av(qc, st_box, *p)
                for pe_ns, fn in fillers:
                    fn()

            # ---- emission schedule ----
            # lead-in: K fully, V chunk 0, Q round 0. The remaining V
            # chunks and Q rounds stream in as attention fillers (their
            # DMAs are issued up front; queues run ahead of the engines).
            for i, (c0, w) in enumerate(kr):
                k_round(c0, w, split=(i == 0))
            xv_tiles = [v_dma(sc) for sc in range(nkc)]
            for sc in range(nkc):
                v_compute(sc, xv_tiles[sc])
            nc.sync.dma_start(out=wq_p.rearrange("p (n j) -> p n j", j=JW),
                              in_=wq.rearrange("(n p) j -> p n j", p=128))
            xq_tiles = {0: xq_round_dma(0)}
            box0 = [None, None]
            for jt in range(2):
                proj_half("q", xq_tiles[0], 512, wq_t, QTt, 0, jt, 0, ND,
                          box0)
            nc.sync.dma_start(out=wo_p.rearrange("p (n j) -> p n j", j=D),
                              in_=wo.rearrange("(n p) j -> p n j", p=128))
            for r in range(1, NQR):
                xq_tiles[r] = xq_round_dma(r)

            def q_round_fillers(r):
                box = [None, None]
                return [
                    (853.0, lambda jt=jt, dlo=dlo: proj_half(
                        "q", xq_tiles[r], 512, wq_t, QTt, r * 512,
                        jt, dlo, dlo + 4, box))
                    for jt in range(2) for dlo in (0, 4)]

            def finish_fillers(qc, on_box):
                # transpose/Wo/out for q chunk qc (already normalized)
                oc_box = {}
                ladder = []
                for qi in range(NQI):
                    ladder.append(
                        (107.0, lambda qi=qi: trans_qi(qc, qi, on_box,
                                                       oc_box)))
                    ladder.append((426.0, lambda qi=qi: wo_qi(qc, qi, 0,
                                                              oc_box)))
                    ladder.append((426.0, lambda qi=qi: wo_qi(qc, qi, 1,
                                                              oc_box)))
                return ladder

            def merge_window(norms, ladder, extras):
                # interleave so every ladder step trails its norm by
                # several tasks (its DVE producer must already be done)
                ex = list(extras)
                lad = list(ladder)
                nor = list(norms)
                fs = []

                def pop(lst):
                    if lst:
                        fs.append(lst.pop(0))
                plan = [nor, ex, nor, ex, lad, nor, ex, lad, nor, lad,
                        ex, lad, lad, ex, lad, lad, ex, lad]
                for lst in plan:
                    pop(lst)
                for lst in (nor, ex, lad):
                    while lst:
                        fs.append(lst.pop(0))
                return fs

            on_prev = None
            qc_prev = None
            for qc in range(NQC):
                extras = []
                if qc + 1 < NQR:
                    extras += q_round_fillers(qc + 1)
                if on_prev is not None:
                    ladder = finish_fillers(qc_prev, on_prev)
                    fillers = merge_window([], ladder, extras)
                else:
                    fillers = extras
                st_box = {"on": [
                    onp.tile([128, JW], BF16, name=f"on{qc}_{qi}", tag="on",
                             bufs=8)
                    for qi in range(NQI)]}
                attention(qc, st_box, fillers)
                on_prev, qc_prev = st_box["on"], qc
            for pe_ns, fn in finish_fillers(qc_prev, on_prev):
                fn()
    nc.compile()
    return nc


def _get_nc(nkc):
    if nkc not in _cached_nc:
        _cached_nc[nkc] = _build(nkc)
    return _cached_nc[nkc]


def kernel(queries, keys, values, valid_lens, Wq, Wk, Wv, Wo, **kwargs):
    queries = np.asarray(queries, dtype=np.float32)
    keys = np.asarray(keys, dtype=np.float32)
    values = np.asarray(values, dtype=np.float32)
    Wq = np.asarray(Wq, dtype=np.float32)
    Wk = np.asarray(Wk, dtype=np.float32)
    Wv = np.asarray(Wv, dtype=np.float32)
    Wo = np.asarray(Wo, dtype=np.float32)
    vls = np.asarray(valid_lens).astype(np.int64)
    B = queries.shape[0]
    assert B == 2 and queries.shape[1:] == (S, D), \
        f"kernel compiled for (2, {S}, {D}), got {queries.shape}"

    bf16 = ml_dtypes.bfloat16
    nkc = int(max(1, -(-int(vls.max()) // 128)))
    nkc = min(nkc, S // 128)
    kw = nkc * 128
    nc = _get_nc(nkc)
    idm = np.eye(128, dtype=bf16)

    in_maps = []
    for b in range(B):
        vl = int(vls[b])
        qb = queries[b]
        if vl <= 0:
            # reference: fully-masked row -> softmax of constant -> uniform.
            qb = np.zeros_like(qb)
            mk = np.zeros(kw, np.float32)
        else:
            mk = np.where(np.arange(kw) < vl, 0.0,
                          MASK_VALUE).astype(np.float32)
        mkt = np.ascontiguousarray(mk.reshape(nkc, 128).T)  # [128, nkc]
        xq = np.ascontiguousarray(qb.T).astype(bf16)
        xk = np.ascontiguousarray(keys[b][:kw].T).astype(bf16)
        xv = np.ascontiguousarray(values[b][:kw].T).astype(bf16)
        for g in range(4):
            in_maps.append({
                "xqT": xq, "xkT": xk, "xvT": xv,
                "wq": np.ascontiguousarray(
                    Wq[:, g * JW:(g + 1) * JW]).astype(bf16),
                "wk": np.ascontiguousarray(
                    Wk[:, g * JW:(g + 1) * JW]).astype(bf16),
                "wv": np.ascontiguousarray(
                    Wv[:, g * JW:(g + 1) * JW]).astype(bf16),
                "wo": np.ascontiguousarray(
                    Wo[g * JW:(g + 1) * JW, :]).astype(bf16),
                "maskb": mkt, "ident": idm,
            })

    res = run_bass_kernel_spmd(nc, in_maps, core_ids=list(range(8)), **kwargs)
    global LAST_RESULTS
    LAST_RESULTS = res

    outp = np.zeros((B, S, D), np.float32)
    for b in range(B):
        acc = res.results[b * 4 + 0]["out"].astype(np.float32)
        for g in range(1, 4):
            acc = acc + res.results[b * 4 + g]["out"].astype(np.float32)
        outp[b] = acc
    return outp


# revision 17
# speedup vs baseline: 2.2001x; 1.0228x over previous
"""Multi-head attention (16 heads, D=1024, B=2, S=2048) on 8 Trainium2 cores.

Sharding: batch (2) x head-groups (4 heads each) = 8 cores, no collectives.
Each core computes, for its batch b and head group g:
  - Q projection for all 2048 positions, K/V projections only for the
    valid-key extent (valid_lens specializes the compiled program: fully
    masked key chunks are never computed - exp() would zero them anyway)
  - per-head attention with masked softmax over the valid chunks only
  - partial output = concat(head outs) @ Wo[rows of group g]
Host sums the 4 per-group partials for each batch.

The program is compiled for NKC = ceil(max(valid_lens)/128) key chunks and
cached per NKC; per-batch masks remain runtime data so one program serves
both batches (a batch with smaller vl just sees more masked columns).

Attention data layout:
  X^T (feature-major, host-transposed) --(Wq/Wk stationary)--> Q^T,K^T [j,s]
  K^T chunk (stationary) x Q^T (moving)  -> scores^T [k,q] in PSUM
  exp(scale*scores + mask) -> E [k,q] bf16
  AV is "flipped" for full PE utilization (128-deep contraction over k,
  128 output partitions over q): stationary E [k,128q] x moving V [k,64j]
  accumulated over k chunks; softmax denominators accumulate in a separate
  PSUM bank via 1-column matmuls against a ones vector.
  Normalize with per-partition reciprocal * scalar-mul, PE-transpose the
  [q,j] tile to [j,q], then Oc^T (stationary) x Wo rows (moving) -> out.

All matmul moving operands are bf16 (1 cycle/row on the PE at any width).

Because engines execute their instruction streams in order, V/Q projection
rounds and the previous q-chunk's normalize/transpose/Wo work are emitted
as filler tasks BETWEEN attention iterations: they fill the PE bubbles the
scores -> exp(ACT) -> AV round trip would otherwise leave.
"""
import ml_dtypes
import numpy as np

import concourse.bacc as bacc
import concourse.mybir as mybir
import concourse.tile as tile
from concourse.bass_utils import run_bass_kernel_spmd

F32 = mybir.dt.float32
BF16 = mybir.dt.bfloat16
AF = mybir.ActivationFunctionType

S = 2048          # sequence length
D = 1024          # model dim
HLOC = 4          # heads per core
HD = 64           # head dim
JW = HLOC * HD    # 256 output dims per core
SCALE = 1.0 / np.sqrt(32.0)   # reference bug: d_k = B*H = 32
MASK_VALUE = -1.0e6

ND = 8            # d chunks of 128 (contraction for projections)
NQR = 4           # q projection rounds of 512
QW = 512          # q chunk width (scores moving width)
NQC = S // QW     # 4 q chunks of 512
NQI = QW // 128   # 4 q subchunks of 128 per q chunk

_cached_nc = {}
LAST_RESULTS = None


def _build(nkc):
    kw = nkc * 128   # padded valid-key extent
    kr = []          # k-projection rounds of up to 512 columns
    c = 0
    while c < kw:
        w = min(512, kw - c)
        kr.append((c, w))
        c += w

    nc = bacc.Bacc("TRN2", target_bir_lowering=False, debug=False,
                   num_swdge_queues=4)

    xqT = nc.dram_tensor("xqT", [D, S], BF16, kind="ExternalInput")
    xkT = nc.dram_tensor("xkT", [D, kw], BF16, kind="ExternalInput")
    xvT = nc.dram_tensor("xvT", [D, kw], BF16, kind="ExternalInput")
    wq = nc.dram_tensor("wq", [D, JW], BF16, kind="ExternalInput")
    wk = nc.dram_tensor("wk", [D, JW], BF16, kind="ExternalInput")
    wv = nc.dram_tensor("wv", [D, JW], BF16, kind="ExternalInput")
    wo = nc.dram_tensor("wo", [JW, D], BF16, kind="ExternalInput")
    maskb = nc.dram_tensor("maskb", [128, nkc], F32, kind="ExternalInput")
    ident = nc.dram_tensor("ident", [128, 128], BF16, kind="ExternalInput")
    out = nc.dram_tensor("out", [S, D], BF16, kind="ExternalOutput")

    with tile.TileContext(nc) as tc:
        with tc.tile_pool(name="wp", bufs=1) as wp, \
             tc.tile_pool(name="per", bufs=1) as per, \
             tc.tile_pool(name="xp", bufs=2) as xp, \
             tc.tile_pool(name="ep", bufs=6) as ep, \
             tc.tile_pool(name="onp", bufs=4) as onp, \
             tc.tile_pool(name="ocp", bufs=8) as ocp, \
             tc.tile_pool(name="rbp", bufs=4) as rbp, \
             tc.tile_pool(name="outp", bufs=6) as outp, \
             tc.tile_pool(name="psc", bufs=4, space="PSUM") as psc, \
             tc.tile_pool(name="pa", bufs=4, space="PSUM") as pa:

            # ---- mask, identity, packed projection weights ----
            mt = wp.tile([128, nkc], F32, name="mt", tag="mt")
            nc.gpsimd.dma_start(out=mt, in_=maskb[:, :])
            idt = wp.tile([128, 128], BF16, name="idt", tag="idt")
            nc.gpsimd.dma_start(out=idt, in_=ident[:, :])
            onest = wp.tile([128, 1], BF16, name="onest", tag="onest")
            nc.vector.memset(onest, 1.0)

            wk_p = wp.tile([128, ND * JW], BF16, name="wk_p", tag="wk_p")
            wq_p = wp.tile([128, ND * JW], BF16, name="wq_p", tag="wq_p")
            wv_p = wp.tile([128, ND * JW], BF16, name="wv_p", tag="wv_p")
            nc.sync.dma_start(out=wk_p.rearrange("p (n j) -> p n j", j=JW),
                              in_=wk.rearrange("(n p) j -> p n j", p=128))
            nc.gpsimd.dma_start(out=wv_p.rearrange("p (n j) -> p n j", j=JW),
                                in_=wv.rearrange("(n p) j -> p n j", p=128))
            wk_t = [wk_p[:, d * JW:(d + 1) * JW] for d in range(ND)]
            wq_t = [wq_p[:, d * JW:(d + 1) * JW] for d in range(ND)]
            wv_t = [wv_p[:, d * JW:(d + 1) * JW] for d in range(ND)]
            wo_p = wp.tile([128, 2 * D], BF16, name="wo_p", tag="wo_p")
            wo_t = [wo_p[:, j * D:(j + 1) * D] for j in range(2)]
            # exp table preload: a 1-element exp so the ACT table load
            # happens during the projection lead-in, not mid-pipeline
            scr1 = wp.tile([1, 1], F32, name="scr1", tag="scr1")
            nc.scalar.activation(scr1, mt[0:1, 0:1], AF.Exp)

            # ---- persistent activations ----
            KTt = [per.tile([128, kw], BF16, name=f"KT{j}", tag=f"KT{j}")
                   for j in range(2)]
            QTt = [per.tile([128, S], BF16, name=f"QT{j}", tag=f"QT{j}")
                   for j in range(2)]
            Vn = [per.tile([128, HLOC * 65], BF16, name=f"Vn{i}",
                           tag=f"Vn{i}") for i in range(nkc)]

            def xq_round_dma(r):
                xt = xp.tile([128, ND * 512], BF16, name=f"xq{r}",
                             tag="xin", bufs=NQR)
                nc.gpsimd.dma_start(
                    out=xt.rearrange("p (n s) -> p n s", s=512),
                    in_=xqT.rearrange("(n p) s -> p n s", p=128)[
                        :, :, r * 512:(r + 1) * 512])
                return xt

            def proj_half(nm, xt, w, wt, OUT, c0, jt, dlo, dhi, pt_box):
                # half of one projection round: d chunks [dlo,dhi) into the
                # jt-th 128-column tile; copies the PSUM out on the last.
                if dlo == 0:
                    pt_box[jt] = psc.tile([128, w], F32,
                                          name=f"p{nm}{c0}_{jt}", tag="ps",
                                          padded_shape=[128, 512])
                pt = pt_box[jt]
                for d in range(dlo, dhi):
                    nc.tensor.matmul(
                        pt, wt[d][:, jt * 128:(jt + 1) * 128],
                        xt[:, d * w:(d + 1) * w],
                        start=(d == 0), stop=(d == ND - 1))
                if dhi == ND:
                    nc.vector.tensor_copy(OUT[jt][:, c0:c0 + w], pt)

            def k_round(c0, w, split=False):
                halves = [(0, w)]
                if split and w > 256:
                    halves = [(0, w // 2), (w // 2, w - w // 2)]
                xts = []
                for (h0, hw) in halves:
                    xt = xp.tile([128, ND * 512], BF16,
                                 name=f"xk{c0}_{h0}", tag="xkin", bufs=2,
                                 padded_shape=[128, ND * 512])
                    nc.sync.dma_start(
                        out=xt[:, 0:ND * hw].rearrange("p (n s) -> p n s",
                                                       s=hw),
                        in_=xkT.rearrange("(n p) s -> p n s", p=128)[
                            :, :, c0 + h0:c0 + h0 + hw])
                    xts.append((h0, hw, xt))
                for (h0, hw, xt) in xts:
                    box = [None, None]
                    for jt in range(2):
                        proj_half("k", xt, hw, wk_t, KTt, c0 + h0, jt,
                                  0, ND, box)

            def v_dma(sc):
                xvt = xp.tile([128, ND * 128], BF16, name=f"xv{sc}",
                              tag="xvin", bufs=nkc)
                nc.gpsimd.dma_start(
                    out=xvt.rearrange("p (n s) -> p n s", s=128),
                    in_=xvT.rearrange("(n p) s -> p n s", p=128)[
                        :, :, sc * 128:(sc + 1) * 128])
                return xvt

            def v_compute(sc, xvt):
                pv = psc.tile([128, JW], F32, name=f"pv{sc}", tag="ps",
                              padded_shape=[128, 512])
                for d in range(ND):
                    nc.tensor.matmul(
                        pv, xvt[:, d * 128:(d + 1) * 128], wv_t[d],
                        start=(d == 0), stop=(d == ND - 1))
                vspl = Vn[sc].rearrange("p (h x) -> p h x", x=65)
                nc.vector.memset(vspl[:, :, 64:65], 1.0)
                nc.vector.tensor_copy(
                    vspl[:, :, 0:64],
                    pv.rearrange("p (h j) -> p h j", j=64))

            def norm_head(qc, h, Ah, on_box):
                # DVE-only: per-partition reciprocal of the denominator
                # column, scale the head's 64 columns to bf16
                for qi in range(NQI):
                    qcg = qc * NQI + qi
                    rt = rbp.tile([128, 1], F32, name=f"rt{qcg}_{h}",
                                  tag="rt")
                    nc.vector.reciprocal(rt, Ah[qi][:, 64:65])
                    nc.vector.tensor_scalar_mul(
                        on_box[qi][:, h * 64:(h + 1) * 64],
                        Ah[qi][:, 0:64], rt)

            def trans_qi(qc, qi, on_box, oc_box):
                # on_box[qi] written per head during attention(qc)
                qcg = qc * NQI + qi
                oc = []
                for jt in range(2):
                    pt = psc.tile([128, 128], BF16, name=f"ptt{qcg}_{jt}",
                                  tag="ps")
                    nc.tensor.transpose(
                        pt, on_box[qi][:, jt * 128:(jt + 1) * 128], idt)
                    ot = ocp.tile([128, 128], BF16, name=f"oc{qcg}_{jt}",
                                  tag="oc")
                    nc.vector.tensor_copy(ot, pt)
                    oc.append(ot)
                oc_box[qi] = oc

            def wo_qi(qc, qi, dh, oc_box):
                qcg = qc * NQI + qi
                pw = psc.tile([128, 512], F32, name=f"pw{qcg}_{dh}",
                              tag="ps")
                for jt in range(2):
                    nc.tensor.matmul(
                        pw, oc_box[qi][jt],
                        wo_t[jt][:, dh * 512:(dh + 1) * 512],
                        start=(jt == 0), stop=(jt == 1))
                ob = outp.tile([128, 512], BF16, name=f"ob{qcg}_{dh}",
                               tag="ob")
                if qcg >= (NQC - 1) * NQI or (qcg + dh) % 4 == 0:
                    nc.scalar.copy(ob, pw)
                else:
                    nc.vector.tensor_copy(ob, pw)
                nc.sync.dma_start(
                    out=out[qcg * 128:(qcg + 1) * 128,
                            dh * 512:(dh + 1) * 512],
                    in_=ob)

            def emit_av(qc, st_box, h, kc, et):
                st = (kc == 0)
                sp = (kc == nkc - 1)
                if st:
                    st_box[h] = [
                        pa.tile([128, 65], F32, name=f"A{qc}_{h}_{qi}",
                                tag="pa") for qi in range(NQI)]
                Ah = st_box[h]
                for qi in range(NQI):
                    nc.tensor.matmul(
                        Ah[qi], et[:, qi * 128:(qi + 1) * 128],
                        Vn[kc][:, h * 65:(h + 1) * 65],
                        start=st, stop=sp)
                if sp:
                    norm_head(qc, h, Ah, st_box["on"])

            def attention(qc, st_box, fillers):
                # iteration order (h, kc); AV(i) is emitted two iterations
                # late so filler PE work and scores(i+1..2) hide exp(i)'s
                # ACT round trip. Fillers are (pe_ns, fn): each iteration
                # pops tasks until the PE has ~the exp() duration queued.
                fillers = list(fillers)
                debt = 0.0
                pending = []
                for h in range(HLOC):
                    jt, hr = divmod(h, 2)
                    hb = hr * 64
                    for kc in range(nkc):
                        pst = psc.tile([128, QW], F32,
                                       name=f"pst{qc}_{h}_{kc}", tag="ps")
                        nc.tensor.matmul(
                            pst,
                            KTt[jt][hb:hb + 64, kc * 128:(kc + 1) * 128],
                            QTt[jt][hb:hb + 64, qc * QW:(qc + 1) * QW],
                            start=True, stop=True)
                        et = ep.tile([128, QW], BF16,
                                     name=f"et{qc}_{h}_{kc}", tag="et")
                        nc.scalar.activation(et, pst, AF.Exp,
                                             bias=mt[:, kc:kc + 1],
                                             scale=float(SCALE))
                        if len(pending) >= 3:
                            emit_av(qc, st_box, *pending.pop(0))
                        debt += 340.0   # exp pacing minus scores+AV work
                        while debt > 0 and fillers:
                            pe_ns, fn = fillers.pop(0)
                            fn()
                            debt -= pe_ns
                        pending.append((h, kc, et))
                for p in pending:
                    emit_av(qc, st_box, *p)
                for pe_ns, fn in fillers:
                    fn()

            # ---- emission schedule ----
            # lead-in: K fully, V chunk 0, Q round 0. The remaining V
            # chunks and Q rounds stream in as attention fillers (their
            # DMAs are issued up front; queues run ahead of the engines).
            for i, (c0, w) in enumerate(kr):
                k_round(c0, w, split=(i == 0))
            xv_tiles = [v_dma(sc) for sc in range(nkc)]
            for sc in range(nkc):
                v_compute(sc, xv_tiles[sc])
            nc.sync.dma_start(out=wq_p.rearrange("p (n j) -> p n j", j=JW),
                              in_=wq.rearrange("(n p) j -> p n j", p=128))
            xq_tiles = {0: xq_round_dma(0)}
            box0 = [None, None]
            for jt in range(2):
                proj_half("q", xq_tiles[0], 512, wq_t, QTt, 0, jt, 0, ND,
                          box0)
            nc.sync.dma_start(out=wo_p.rearrange("p (n j) -> p n j", j=D),
                              in_=wo.rearrange("(n p) j -> p n j", p=128))
            for r in range(1, NQR):
                xq_tiles[r] = xq_round_dma(r)

            def q_round_fillers(r):
                box = [None, None]
                return [
                    (853.0, lambda jt=jt, dlo=dlo: proj_half(
                        "q", xq_tiles[r], 512, wq_t, QTt, r * 512,
                        jt, dlo, dlo + 4, box))
                    for jt in range(2) for dlo in (0, 4)]

            def finish_fillers(qc, on_box):
                # transpose/Wo/out for q chunk qc (already normalized)
                oc_box = {}
                ladder = []
                for qi in range(NQI):
                    ladder.append(
                        (107.0, lambda qi=qi: trans_qi(qc, qi, on_box,
                                                       oc_box)))
                    ladder.append((426.0, lambda qi=qi: wo_qi(qc, qi, 0,
                                                              oc_box)))
                    ladder.append((426.0, lambda qi=qi: wo_qi(qc, qi, 1,
                                                              oc_box)))
                return ladder

            def merge_window(norms, ladder, extras):
                # interleave so every ladder step trails its norm by
                # several tasks (its DVE producer must already be done)
                ex = list(extras)
                lad = list(ladder)
                nor = list(norms)
                fs = []

                def pop(lst):
                    if lst:
                        fs.append(lst.pop(0))
                plan = [nor, ex, nor, ex, lad, nor, ex, lad, nor, lad,
                        ex, lad, lad, ex, lad, lad, ex, lad]
                for lst in plan:
                    pop(lst)
                for lst in (nor, ex, lad):
                    while lst:
                        fs.append(lst.pop(0))
                return fs

            on_prev = None
            qc_prev = None
            for qc in range(NQC):
                extras = []
                if qc + 1 < NQR:
                    extras += q_round_fillers(qc + 1)
                if on_prev is not None:
                    ladder = finish_fillers(qc_prev, on_prev)
                    fillers = merge_window([], ladder, extras)
                else:
                    fillers = extras
                st_box = {"on": [
                    onp.tile([128, JW], BF16, name=f"on{qc}_{qi}", tag="on",
                             bufs=8)
                    for qi in range(NQI)]}
                attention(qc, st_box, fillers)
                on_prev, qc_prev = st_box["on"], qc
            for pe_ns, fn in finish_fillers(qc_prev, on_prev):
                fn()
    nc.compile()
    return nc


def _get_nc(nkc):
    if nkc not in _cached_nc:
        _cached_nc[nkc] = _build(nkc)
    return _cached_nc[nkc]


def kernel(queries, keys, values, valid_lens, Wq, Wk, Wv, Wo, **kwargs):
    queries = np.asarray(queries, dtype=np.float32)
    keys = np.asarray(keys, dtype=np.float32)
    values = np.asarray(values, dtype=np.float32)
    Wq = np.asarray(Wq, dtype=np.float32)
    Wk = np.asarray(Wk, dtype=np.float32)
    Wv = np.asarray(Wv, dtype=np.float32)
    Wo = np.asarray(Wo, dtype=np.float32)
    vls = np.asarray(valid_lens).astype(np.int64)
    B = queries.shape[0]
    assert B == 2 and queries.shape[1:] == (S, D), \
        f"kernel compiled for (2, {S}, {D}), got {queries.shape}"

    bf16 = ml_dtypes.bfloat16
    nkc = int(max(1, -(-int(vls.max()) // 128)))
    nkc = min(nkc, S // 128)
    kw = nkc * 128
    nc = _get_nc(nkc)
    idm = np.eye(128, dtype=bf16)

    in_maps = []
    for b in range(B):
        vl = int(vls[b])
        qb = queries[b]
        if vl <= 0:
            # reference: fully-masked row -> softmax of constant -> uniform.
            qb = np.zeros_like(qb)
            mk = np.zeros(kw, np.float32)
        else:
            mk = np.where(np.arange(kw) < vl, 0.0,
                          MASK_VALUE).astype(np.float32)
        mkt = np.ascontiguousarray(mk.reshape(nkc, 128).T)  # [128, nkc]
        xq = np.ascontiguousarray(qb.T).astype(bf16)
        xk = np.ascontiguousarray(keys[b][:kw].T).astype(bf16)
        xv = np.ascontiguousarray(values[b][:kw].T).astype(bf16)
        for g in range(4):
            in_maps.append({
                "xqT": xq, "xkT": xk, "xvT": xv,
                "wq": np.ascontiguousarray(
                    Wq[:, g * JW:(g + 1) * JW]).astype(bf16),
                "wk": np.ascontiguousarray(
                    Wk[:, g * JW:(g + 1) * JW]).astype(bf16),
                "wv": np.ascontiguousarray(
                    Wv[:, g * JW:(g + 1) * JW]).astype(bf16),
                "wo": np.ascontiguousarray(
                    Wo[g * JW:(g + 1) * JW, :]).astype(bf16),
                "maskb": mkt, "ident": idm,
            })

    res = run_bass_kernel_spmd(nc, in_maps, core_ids=list(range(8)), **kwargs)
    global LAST_RESULTS
    LAST_RESULTS = res

    outp = np.zeros((B, S, D), np.float32)
    for b in range(B):
        acc = res.results[b * 4 + 0]["out"].astype(np.float32)
        for g in range(1, 4):
            acc = acc + res.results[b * 4 + g]["out"].astype(np.float32)
        outp[b] = acc
    return outp
